# revision 9
# baseline (speedup 1.0000x reference)
"""MeanAggregator (GNN message passing) Trainium2 Bass kernel.

out[n, :] = mean_k features[neigh_idx[n, k], :]
N=100000, K=6, V=200000, D=128, f32.

Strategy: shard target nodes across 8 cores (12500 each), replicate the
feature table.  Per core, nodes are processed in tiles of 128 (one node
per SBUF partition).  A group of G node-tiles is gathered with a single
gpsimd indirect DMA (G*K rows per partition, 512B each), the K-mean is a
single strided DVE tensor_reduce (features are pre-scaled by 1/6 on the
host so sum == mean), and the result is written back with one HWDGE DMA.
"""

import numpy as np

import concourse.bass as bass
import concourse.bacc as bacc
import concourse.mybir as mybir
import concourse.tile as tile
from concourse.bass_utils import run_bass_kernel_spmd


def _legalize_waits(nc):
    """This container's walrus allows at most ONE sync wait per
    instruction ("Too many sync wait commands").  Tile freely attaches
    several.  Rewrite: for each instruction with k>1 waits, hoist k-1 of
    them onto fresh same-engine nops placed immediately before it —
    semantically identical for in-order engine queues."""
    import bass_rust
    cnt = 0
    for f in nc.m.functions:
        for bb in f.blocks:
            out = []
            changed = False
            for inst in bb.instructions:
                si = inst.sync_info
                waits = list(si.on_wait) if si is not None and si.on_wait else []
                if len(waits) > 1:
                    ups = list(si.on_update) if si.on_update else []
                    for w in waits[:-1]:
                        n = bass_rust.InstNoOp(name=f"waitsplit_{cnt}")
                        cnt += 1
                        n.engine = inst.engine
                        n.sync_info = mybir.SyncInfo(on_wait=[w], on_update=[])
                        out.append(n)
                    inst.sync_info = mybir.SyncInfo(
                        on_wait=[waits[-1]], on_update=ups)
                    changed = True
                out.append(inst)
            if changed:
                bb.instructions = out
    return cnt

N = 100000
K = 6
V = 200000
D = 128
NCORES = 8
P = 128
NPC = N // NCORES            # 12500 nodes per core
TILES = -(-NPC // P)         # 98 node-tiles per core
NPAD = TILES * P             # 12544 padded nodes per core
GROUP = 7                    # node-tiles per indirect-gather DMA
NGROUPS = TILES // GROUP     # 14


def build_nc(v=V, tiles=TILES, group=14, gather_bufs=12, acc_bufs=3, nq=4):
    """One indirect DMA per (node-tile, k): gathers 128 rows (one per
    partition; HW consumes exactly one index per partition).  Outputs are
    staged in groups of `group` node-tiles so each out-DMA moves
    per-partition-contiguous group*512B chunks."""
    assert tiles % group == 0
    nc = bacc.Bacc("TRN2", target_bir_lowering=False, num_swdge_queues=nq)
    feat = nc.dram_tensor("features", [v, D], mybir.dt.float32,
                          kind="ExternalInput")
    idx = nc.dram_tensor("idx", [P, tiles * K], mybir.dt.int32,
                         kind="ExternalInput")
    out = nc.dram_tensor("out", [P, tiles * D], mybir.dt.float32,
                         kind="ExternalOutput")
    with tile.TileContext(nc) as tc:
        with tc.tile_pool(name="idxp", bufs=1) as idxp, \
             tc.tile_pool(name="gat", bufs=gather_bufs) as gat, \
             tc.tile_pool(name="accp", bufs=acc_bufs) as accp:
            idx_t = idxp.tile([P, tiles * K], mybir.dt.int32)
            nc.sync.dma_start(out=idx_t[:], in_=idx[:])
            for g in range(tiles // group):
                acc = accp.tile([P, group * D], mybir.dt.float32)
                for tl in range(group):
                    t = g * group + tl
                    gt = gat.tile([P, K * D], mybir.dt.float32)
                    for k in range(K):
                        bi = nc.gpsimd.indirect_dma_start(
                            out=gt[:, k * D:(k + 1) * D],
                            out_offset=None,
                            in_=feat[:],
                            in_offset=bass.IndirectOffsetOnAxis(
                                ap=idx_t[:, t * K + k:t * K + k + 1],
                                axis=0,
                            ),
                        )
                        q = (t * K + k) % nq
                        if q:
                            bi.ins.queue = f"qPoolDynamic{q}"
                    nc.vector.tensor_reduce(
                        out=acc[:, tl * D:(tl + 1) * D],
                        in_=gt[:].rearrange("p (k d) -> p d k", k=K, d=D),
                        axis=mybir.AxisListType.X,
                        op=mybir.AluOpType.add,
                    )
                nc.sync.dma_start(
                    out=out[:, g * group * D:(g + 1) * group * D],
                    in_=acc[:],
                )
    nc.compile()
    return nc


_nc_cache = {}


def _get_nc():
    if "nc" not in _nc_cache:
        _nc_cache["nc"] = build_nc()
    return _nc_cache["nc"]


def _prep_idx(neigh_core: np.ndarray) -> np.ndarray:
    """[NPC, K] int -> [P, TILES*K] int32 laid out so that
    prep[p, t*K + k] = neigh_core[t*P + p, k] (pad nodes gather row 0)."""
    sp = np.zeros((NPAD, K), np.int32)
    sp[:NPC] = neigh_core
    return np.ascontiguousarray(
        sp.reshape(TILES, P, K).transpose(1, 0, 2).reshape(P, TILES * K))


def make_in_maps(features: np.ndarray, neigh_idx: np.ndarray):
    feat = (np.asarray(features, dtype=np.float32) *
            np.float32(1.0 / K))
    feat = np.ascontiguousarray(feat)
    ni = np.asarray(neigh_idx).astype(np.int32).reshape(NCORES, NPC, K)
    return [{"features": feat, "idx": _prep_idx(ni[c])}
            for c in range(NCORES)]


def assemble_out(results) -> np.ndarray:
    outs = []
    for c in range(NCORES):
        o = results[c]["out"]
        o = o.reshape(P, TILES, D).transpose(1, 0, 2).reshape(NPAD, D)[:NPC]
        outs.append(o)
    return np.ascontiguousarray(np.concatenate(outs, axis=0))


def kernel(features: np.ndarray, neigh_idx: np.ndarray, **run_kwargs):
    in_maps = make_in_maps(features, neigh_idx)
    res = run_bass_kernel_spmd(_get_nc(), in_maps,
                               core_ids=list(range(NCORES)), **run_kwargs)
    full = assemble_out(res.results)
    if run_kwargs:
        return full, res
    return full
